# revision 9
# baseline (speedup 1.0000x reference)
"""Trainium2 Bass kernel for ChannelAttention-SNN (LIF -> GAP -> 1x1conv -> BN
-> 1x1conv -> BN).

Contract: kernel(**inputs) takes the FULL unsharded inputs (as produced by
setup_inputs) and returns the FULL output [T, B, C, 1] float32.

Strategy (hardcoded for T=4, B=16, C=512, N=1024, Cr=64, 8 cores):
  - Data-parallel over B: core m processes b in {2m, 2m+1}.
  - LIF scan is unrolled over T in "P-space": P_t = 2^t * v_pre_t, so
      P_t = P_{t-1} * m_{t-1} + 2^{t-1} * x_t,   spike_t <=> P_t >= 2^t,
    which folds the 1/tau decay into the (free) scale of the ScalarE cast
    fp32->bf16. Per timestep the VectorE does: mask = (P < theta) with a
    fused free-dim count (accum_out) that directly yields the GAP sums,
    a mask multiply, and an add.
  - The conv/BN tail runs in fp32 on the PE/DVE: per-core h1 partial rows
    [8, 64], one AllGather, then every core redundantly computes the
    batch-norm tail for all 64 rows and writes the full output (batch-stat
    all-reduce is subsumed by the gather; outputs are identical across
    cores).
"""

import numpy as np

import concourse.bacc as bacc
import concourse.bass as bass
import concourse.mybir as mybir
import concourse.tile as tile
from concourse.bass_utils import run_bass_kernel_spmd
from concourse.masks import make_identity

T, B, C, N, CR = 4, 16, 512, 1024, 64
NCORES = 8
BL = B // NCORES            # batch rows per core (2)
CB = C // 128               # 128-partition channel blocks (4)
ROWS = T * BL               # local (t, b) rows (8)
TBALL = T * B               # total batch rows for BN (64)
BN_EPS = 1e-5

F32 = mybir.dt.float32
BF16 = mybir.dt.bfloat16
OP = mybir.AluOpType
AF = mybir.ActivationFunctionType
AX = mybir.AxisListType


def _emit(tc, ctx):
    nc = tc.nc
    x = nc.dram_tensor("x", [T, BL, C, N], F32, kind="ExternalInput").ap()
    w1t = nc.dram_tensor("w1t", [C, CR], F32, kind="ExternalInput").ap()
    w2t = nc.dram_tensor("w2t", [CR, C], F32, kind="ExternalInput").ap()
    g1 = nc.dram_tensor("gamma1", [CR, 1], F32, kind="ExternalInput").ap()
    be1 = nc.dram_tensor("beta1", [CR, 1], F32, kind="ExternalInput").ap()
    g2 = nc.dram_tensor("gamma2", [128, CB], F32, kind="ExternalInput").ap()
    be2 = nc.dram_tensor("beta2", [128, CB], F32, kind="ExternalInput").ap()
    out = nc.dram_tensor("out", [T, B, C], F32, kind="ExternalOutput").ap()

    consts = ctx.enter_context(tc.tile_pool(name="consts", bufs=1))
    xpool = ctx.enter_context(tc.tile_pool(name="xp", bufs=5))
    ypool = ctx.enter_context(tc.tile_pool(name="yp", bufs=3))
    qpool = ctx.enter_context(tc.tile_pool(name="qp", bufs=3))
    mpool = ctx.enter_context(tc.tile_pool(name="mp", bufs=8))
    spool = ctx.enter_context(tc.tile_pool(name="sp", bufs=1))
    tpool = ctx.enter_context(tc.tile_pool(name="tp", bufs=2))
    psum = ctx.enter_context(tc.tile_pool(name="ps", bufs=1, space="PSUM"))
    psum2 = ctx.enter_context(tc.tile_pool(name="ps2", bufs=2, space="PSUM"))
    dram = ctx.enter_context(tc.tile_pool(name="dr", bufs=1, space="DRAM"))

    # ---- constants / weights (overlaps with the streaming phase) ----
    ident = consts.tile([128, 128], F32)
    make_identity(nc, ident)
    w1t_sb = consts.tile([128, CB, CR], F32)
    for cb in range(CB):
        nc.sync.dma_start(w1t_sb[:, cb, :], w1t[cb * 128:(cb + 1) * 128, :])
    w2t_sb = consts.tile([CR, C], F32)
    nc.sync.dma_start(w2t_sb[:], w2t[:])
    g1_sb = consts.tile([CR, 1], F32)
    nc.sync.dma_start(g1_sb[:], g1[:])
    be1_sb = consts.tile([CR, 1], F32)
    nc.sync.dma_start(be1_sb[:], be1[:])
    g2_sb = consts.tile([128, CB], F32)
    nc.sync.dma_start(g2_sb[:], g2[:])
    be2_sb = consts.tile([128, CB], F32)
    nc.sync.dma_start(be2_sb[:], be2[:])
    ones_sb = consts.tile([TBALL, 1], F32)
    nc.vector.memset(ones_sb[:], 1.0)
    eps_sb = consts.tile([128, 1], F32)
    nc.vector.memset(eps_sb[:], BN_EPS)

    # ---- streaming LIF + GAP ----
    # stats[:, cb, t, b] = sum_n (P_t < theta_t)  (count of NON-spikes)
    stats = spool.tile([128, CB, T, BL], F32)
    pstate = [spool.tile([128, BL, N], BF16, tag=f"P{cb}", name=f"P{cb}")
              for cb in range(CB)]
    masks = [None] * CB

    for t in range(T):
        for cb in range(CB):
            P = pstate[cb]
            xt = xpool.tile([128, BL, N], F32)
            src = x[t, :, cb * 128:(cb + 1) * 128, :].rearrange("b c n -> c b n")
            nc.sync.dma_start(xt[:], src)
            if t == 0:
                # P_1 = x_1 (cast to bf16)
                nc.scalar.activation(P[:], xt[:], AF.Copy, scale=1.0)
            else:
                y = ypool.tile([128, BL, N], BF16)
                nc.scalar.activation(y[:], xt[:], AF.Copy, scale=float(2 ** t))
                q = qpool.tile([128, BL, N], BF16)
                nc.vector.tensor_mul(q[:], P[:], masks[cb][:])
                nc.vector.tensor_add(P[:], q[:], y[:])
            m = mpool.tile([128, BL, N], BF16)
            theta = float(2 ** (t + 1))
            for b in range(BL):
                nc.vector.tensor_scalar(
                    out=m[:, b, :],
                    in0=P[:, b, :],
                    scalar1=theta,
                    scalar2=None,
                    op0=OP.is_lt,
                    op1=OP.add,
                    accum_out=stats[:, cb, t, b:b + 1],
                )
            masks[cb] = m

    # ---- g = 1 - stats/N ; h1 partial rows = g @ w1.T  (per-core rows) ----
    gm = spool.tile([128, CB, T, BL], F32)
    nc.vector.tensor_scalar(
        out=gm[:], in0=stats[:], scalar1=-1.0 / N, scalar2=1.0,
        op0=OP.mult, op1=OP.add,
    )
    h1_ps = psum.tile([ROWS, CR], F32, tag="h1")
    for cb in range(CB):
        nc.tensor.matmul(
            h1_ps[:],
            gm[:, cb].rearrange("p t b -> p (t b)"),
            w1t_sb[:, cb, :],
            start=(cb == 0),
            stop=(cb == CB - 1),
        )
    h1_sb = tpool.tile([ROWS, CR], F32, tag="h1s")
    nc.vector.tensor_copy(h1_sb[:], h1_ps[:])

    # ---- AllGather local h1 rows -> all 64 batch rows on every core ----
    cc_in = dram.tile([ROWS, CR], F32)
    cc_out = dram.tile([TBALL, CR], F32)
    nc.sync.dma_start(cc_in[:], h1_sb[:])
    nc.gpsimd.collective_compute(
        "AllGather", OP.bypass,
        replica_groups=[list(range(NCORES))],
        ins=[cc_in[:].opt()], outs=[cc_out[:].opt()],
    )
    h1_all = tpool.tile([TBALL, CR], F32, tag="h1a")
    nc.sync.dma_start(h1_all[:], cc_out[:])

    # ---- BN1 (stats over the 64 batch rows), in [j, tb] layout ----
    h1T_ps = psum.tile([CR, TBALL], F32, tag="tr")
    nc.tensor.transpose(h1T_ps[:], h1_all[:], ident[:TBALL, :TBALL])
    h1T = tpool.tile([CR, TBALL], F32, tag="h1T")
    nc.vector.tensor_copy(h1T[:], h1T_ps[:])

    s1 = tpool.tile([CR, 1], F32, tag="s1")
    nc.vector.reduce_sum(s1[:], h1T[:], axis=AX.X)
    sq1 = tpool.tile([CR, TBALL], F32, tag="sq1")
    nc.vector.tensor_mul(sq1[:], h1T[:], h1T[:])
    q1 = tpool.tile([CR, 1], F32, tag="q1")
    nc.vector.reduce_sum(q1[:], sq1[:], axis=AX.X)
    mu1 = tpool.tile([CR, 1], F32, tag="mu1")
    nc.vector.tensor_scalar_mul(mu1[:], s1[:], 1.0 / TBALL)
    var1 = tpool.tile([CR, 1], F32, tag="var1")
    nc.vector.tensor_scalar_mul(var1[:], q1[:], 1.0 / TBALL)
    musq1 = tpool.tile([CR, 1], F32, tag="musq1")
    nc.vector.tensor_mul(musq1[:], mu1[:], mu1[:])
    nc.vector.tensor_sub(var1[:], var1[:], musq1[:])
    std1 = tpool.tile([CR, 1], F32, tag="std1")
    nc.scalar.activation(std1[:], var1[:], AF.Sqrt, bias=eps_sb[:CR])
    rstd1 = tpool.tile([CR, 1], F32, tag="rstd1")
    nc.vector.reciprocal(rstd1[:], std1[:])
    d1 = tpool.tile([CR, 1], F32, tag="d1")
    nc.vector.tensor_mul(d1[:], rstd1[:], g1_sb[:])
    sh1 = tpool.tile([CR, 1], F32, tag="sh1")
    nc.vector.tensor_mul(sh1[:], mu1[:], d1[:])
    nc.vector.tensor_sub(sh1[:], be1_sb[:], sh1[:])
    h1nT = tpool.tile([CR, TBALL], F32, tag="h1nT")
    nc.vector.tensor_scalar(
        out=h1nT[:], in0=h1T[:], scalar1=d1[:], scalar2=sh1[:],
        op0=OP.mult, op1=OP.add,
    )

    # ---- h2 = h1n @ w2.T  -> [64 rows, 512 ch] ----
    h2_ps = psum.tile([TBALL, C], F32, tag="h2")
    nc.tensor.matmul(h2_ps[:], h1nT[:], w2t_sb[:], start=True, stop=True)
    h2 = tpool.tile([TBALL, C], F32, tag="h2s")
    nc.vector.tensor_copy(h2[:], h2_ps[:])
    h2sq = tpool.tile([TBALL, C], F32, tag="h2sq")
    nc.vector.tensor_mul(h2sq[:], h2[:], h2[:])

    # ---- BN2 channel stats in transposed [128, CB] layout ----
    s2_ps = psum2.tile([128, CB], F32, tag="s2")
    q2_ps = psum2.tile([128, CB], F32, tag="q2")
    for cb in range(CB):
        nc.tensor.matmul(s2_ps[:, cb:cb + 1],
                         h2[:, cb * 128:(cb + 1) * 128], ones_sb[:],
                         start=True, stop=True)
        nc.tensor.matmul(q2_ps[:, cb:cb + 1],
                         h2sq[:, cb * 128:(cb + 1) * 128], ones_sb[:],
                         start=True, stop=True)
    mu2 = tpool.tile([128, CB], F32, tag="mu2")
    nc.vector.tensor_scalar_mul(mu2[:], s2_ps[:], 1.0 / TBALL)
    var2 = tpool.tile([128, CB], F32, tag="var2")
    nc.vector.tensor_scalar_mul(var2[:], q2_ps[:], 1.0 / TBALL)
    musq2 = tpool.tile([128, CB], F32, tag="musq2")
    nc.vector.tensor_mul(musq2[:], mu2[:], mu2[:])
    nc.vector.tensor_sub(var2[:], var2[:], musq2[:])
    std2 = tpool.tile([128, CB], F32, tag="std2")
    nc.scalar.activation(std2[:], var2[:], AF.Sqrt, bias=eps_sb[:])
    rstd2 = tpool.tile([128, CB], F32, tag="rstd2")
    nc.vector.reciprocal(rstd2[:], std2[:])
    d2 = tpool.tile([128, CB], F32, tag="d2")
    nc.vector.tensor_mul(d2[:], rstd2[:], g2_sb[:])
    sh2 = tpool.tile([128, CB], F32, tag="sh2")
    nc.vector.tensor_mul(sh2[:], mu2[:], d2[:])
    nc.vector.tensor_sub(sh2[:], be2_sb[:], sh2[:])

    # bounce the per-channel affine params through DRAM to broadcast them
    # across the 64 row-partitions: pair_dram[k, c] with c = cb*128 + p.
    pair_dram = dram.tile([2, C], F32)
    nc.sync.dma_start(
        pair_dram[0:1, :].rearrange("one (cb p) -> (one p) cb", p=128), d2[:])
    nc.sync.dma_start(
        pair_dram[1:2, :].rearrange("one (cb p) -> (one p) cb", p=128), sh2[:])
    pairs = tpool.tile([TBALL, 2, C], F32, tag="pairs")
    pd = pair_dram[:]
    bcast = bass.AP(tensor=pd.tensor, offset=pd.offset,
                    ap=[[0, TBALL]] + list(pd.ap))
    nc.sync.dma_start(pairs[:], bcast)

    outf = tpool.tile([TBALL, C], F32, tag="outf")
    nc.vector.tensor_mul(outf[:], h2[:], pairs[:, 0, :])
    nc.vector.tensor_add(outf[:], outf[:], pairs[:, 1, :])

    # rows are ordered (core, t, b_local); write every core's full output.
    for mcore in range(NCORES):
        # dst traversal order (t, b, c) matches src row order (t*BL+b, c);
        # dma_start only requires equal total sizes.
        dst = out[:, BL * mcore:BL * (mcore + 1), :]
        nc.sync.dma_start(dst, outf[ROWS * mcore:ROWS * (mcore + 1), :])


_CACHE = {}


def _build():
    if "nc" in _CACHE:
        return _CACHE["nc"]
    from contextlib import ExitStack
    nc = bacc.Bacc("TRN2", target_bir_lowering=False, debug=False,
                   num_devices=NCORES)
    with tile.TileContext(nc) as tc, ExitStack() as ctx:
        _emit(tc, ctx)
    nc.compile()
    _CACHE["nc"] = nc
    return nc


def make_in_maps(x, w1, gamma1, beta1, w2, gamma2, beta2):
    x = np.ascontiguousarray(np.asarray(x, dtype=np.float32))
    w1t = np.ascontiguousarray(np.asarray(w1, np.float32).T)
    w2t = np.ascontiguousarray(np.asarray(w2, np.float32).T)
    g1 = np.asarray(gamma1, np.float32).reshape(CR, 1)
    be1 = np.asarray(beta1, np.float32).reshape(CR, 1)
    # channel c = cb*128 + p  ->  [p, cb] layout
    g2 = np.ascontiguousarray(np.asarray(gamma2, np.float32).reshape(CB, 128).T)
    be2 = np.ascontiguousarray(np.asarray(beta2, np.float32).reshape(CB, 128).T)
    return [
        {
            "x": np.ascontiguousarray(x[:, BL * m:BL * (m + 1)]),
            "w1t": w1t, "w2t": w2t,
            "gamma1": g1, "beta1": be1,
            "gamma2": g2, "beta2": be2,
        }
        for m in range(NCORES)
    ]


def kernel(x, w1, b1, gamma1, beta1, w2, b2, gamma2, beta2):
    # b1/b2 cancel exactly inside the following batch-norms; unused.
    nc = _build()
    in_maps = make_in_maps(x, w1, gamma1, beta1, w2, gamma2, beta2)
    res = run_bass_kernel_spmd(nc, in_maps, core_ids=list(range(NCORES)))
    out = res.results[0]["out"]
    return np.asarray(out, np.float32).reshape(T, B, C, 1)


# revision 13
# speedup vs baseline: 58.8671x; 58.8671x over previous
"""Trainium2 Bass kernel for ChannelAttention-SNN (LIF -> GAP -> 1x1conv -> BN
-> 1x1conv -> BN).

Contract: kernel(**inputs) takes the FULL unsharded inputs (as produced by
setup_inputs) and returns the FULL output [T, B, C, 1] float32.

Strategy (hardcoded for T=4, B=16, C=512, N=1024, Cr=64, 8 cores):
  - Data-parallel over B: core m processes b in {2m, 2m+1}.
  - LIF scan is unrolled over T in "P-space": P_t = 2^t * v_pre_t, so
      P_t = P_{t-1} * m_{t-1} + 2^{t-1} * x_t,   spike_t <=> P_t >= 2^t,
    which folds the 1/tau decay into the (free) scale of the ScalarE cast
    fp32->bf16. Per timestep the VectorE does: mask = (P < theta) with a
    fused free-dim count (accum_out) that directly yields the GAP sums,
    a mask multiply, and an add.
  - The conv/BN tail runs in fp32 on the PE/DVE: per-core h1 partial rows
    [8, 64], one AllGather, then every core redundantly computes the
    batch-norm tail for all 64 rows and writes the full output (batch-stat
    all-reduce is subsumed by the gather; outputs are identical across
    cores).
"""

import numpy as np

import concourse.bacc as bacc
import concourse.bass as bass
import concourse.mybir as mybir
import concourse.tile as tile
from concourse.bass_utils import run_bass_kernel_spmd
from concourse.masks import make_identity

T, B, C, N, CR = 4, 16, 512, 1024, 64
NCORES = 8
BL = B // NCORES            # batch rows per core (2)
CB = C // 128               # 128-partition channel blocks (4)
ROWS = T * BL               # local (t, b) rows (8)
TBALL = T * B               # total batch rows for BN (64)
BN_EPS = 1e-5

F32 = mybir.dt.float32
BF16 = mybir.dt.bfloat16
OP = mybir.AluOpType
AF = mybir.ActivationFunctionType
AX = mybir.AxisListType


def _emit(tc, ctx, repeat=1, tail_repeat=1, single=False, tail_stage=99):
    nc = tc.nc
    x = nc.dram_tensor("x", [T, BL, C, N], F32, kind="ExternalInput").ap()
    w1t = nc.dram_tensor("w1t", [C, CR], F32, kind="ExternalInput").ap()
    w2t = nc.dram_tensor("w2t", [CR, C], F32, kind="ExternalInput").ap()
    g1 = nc.dram_tensor("gamma1", [CR, 1], F32, kind="ExternalInput").ap()
    be1 = nc.dram_tensor("beta1", [CR, 1], F32, kind="ExternalInput").ap()
    g2 = nc.dram_tensor("gamma2", [1, C], F32, kind="ExternalInput").ap()
    be2 = nc.dram_tensor("beta2", [1, C], F32, kind="ExternalInput").ap()
    out = nc.dram_tensor("out", [T, B, C], F32, kind="ExternalOutput").ap()

    consts = ctx.enter_context(tc.tile_pool(name="consts", bufs=1))
    xpool = ctx.enter_context(tc.tile_pool(name="xp", bufs=5))
    ypool = ctx.enter_context(tc.tile_pool(name="yp", bufs=3))
    qpool = ctx.enter_context(tc.tile_pool(name="qp", bufs=3))
    mpool = ctx.enter_context(tc.tile_pool(name="mp", bufs=8))
    spool = ctx.enter_context(tc.tile_pool(name="sp", bufs=1))
    tpool = ctx.enter_context(tc.tile_pool(name="tp", bufs=2))
    psum = ctx.enter_context(tc.tile_pool(name="ps", bufs=1, space="PSUM"))
    psum2 = ctx.enter_context(tc.tile_pool(name="ps2", bufs=1, space="PSUM"))
    dram = ctx.enter_context(tc.tile_pool(name="dr", bufs=1, space="DRAM"))

    # ---- constants / weights (overlaps with the streaming phase) ----
    ident = consts.tile([128, 128], F32)
    make_identity(nc, ident)
    w1t_sb = consts.tile([128, CB, CR], F32)
    for cb in range(CB):
        nc.sync.dma_start(w1t_sb[:, cb, :], w1t[cb * 128:(cb + 1) * 128, :])
    w2t_sb = consts.tile([CR, C], F32)
    nc.sync.dma_start(w2t_sb[:], w2t[:])
    g1_sb = consts.tile([CR, 1], F32)
    nc.sync.dma_start(g1_sb[:], g1[:])
    be1_sb = consts.tile([CR, 1], F32)
    nc.sync.dma_start(be1_sb[:], be1[:])
    g2_sb = consts.tile([1, C], F32)
    nc.sync.dma_start(g2_sb[:], g2[:])
    be2_sb = consts.tile([1, C], F32)
    nc.sync.dma_start(be2_sb[:], be2[:])
    # BN1 guarantees mean(h1n) == beta1, so BN2's channel mean is known
    # ahead of time: mu2 = beta1 @ w2.T (+b2, which cancels).
    mu2_ps = psum2.tile([1, C], F32, tag="mu2p", name="mu2_ps")
    nc.tensor.matmul(mu2_ps[:], be1_sb[:], w2t_sb[:], start=True, stop=True)
    mu2row = consts.tile([1, C], F32)
    nc.vector.tensor_scalar_mul(mu2row[:], mu2_ps[:], 1.0 / 1.0)
    mu2sq = consts.tile([1, C], F32)
    nc.vector.tensor_mul(mu2sq[:], mu2row[:], mu2row[:])
    mu2b = consts.tile([TBALL, C], F32)
    nc.gpsimd.partition_broadcast(mu2b[:], mu2row[:])
    be2b = consts.tile([TBALL, C], F32)
    nc.gpsimd.partition_broadcast(be2b[:], be2_sb[:])
    ones_sb = consts.tile([TBALL, 1], F32)
    nc.vector.memset(ones_sb[:], 1.0)
    eps_sb = consts.tile([128, 1], F32)
    nc.vector.memset(eps_sb[:], BN_EPS)
    warm_sb = consts.tile([128, 1], F32)
    # warm the Sqrt activation table during the streaming phase
    nc.scalar.activation(warm_sb[:], eps_sb[:], AF.Sqrt, bias=eps_sb[:])
    ones_bf = consts.tile([TBALL, 1], BF16)
    nc.vector.memset(ones_bf[:], 1.0)

    # ---- streaming LIF + GAP ----
    # stats[:, cb, t, b] = sum_n (P_t < theta_t)  (count of NON-spikes)
    stats = spool.tile([128, CB, T, BL], F32)
    pstate = [spool.tile([128, BL, N], BF16, tag=f"P{cb}", name=f"P{cb}")
              for cb in range(CB)]
    masks = [None] * CB

    for _rep in range(repeat):
      for t in range(T):
          for cb in range(CB):
              P = pstate[cb]
              xt = xpool.tile([128, BL, N], F32)
              src = x[t, :, cb * 128:(cb + 1) * 128, :].rearrange("b c n -> c b n")
              nc.sync.dma_start(xt[:], src)
              if t == 0:
                  # P_1 = x_1 (cast to bf16)
                  nc.scalar.activation(P[:], xt[:], AF.Copy, scale=1.0)
              else:
                  y = ypool.tile([128, BL, N], BF16)
                  nc.scalar.activation(y[:], xt[:], AF.Copy, scale=float(2 ** t))
                  q = qpool.tile([128, BL, N], BF16)
                  nc.vector.tensor_mul(q[:], P[:], masks[cb][:])
                  nc.vector.tensor_add(P[:], q[:], y[:])
              m = mpool.tile([128, BL, N], BF16)
              theta = float(2 ** (t + 1))
              for b in range(BL):
                  nc.vector.tensor_scalar(
                      out=m[:, b, :],
                      in0=P[:, b, :],
                      scalar1=theta,
                      scalar2=None,
                      op0=OP.is_lt,
                      op1=OP.add,
                      accum_out=stats[:, cb, t, b:b + 1],
                  )
              masks[cb] = m

    # ---- g = 1 - stats/N ; h1 partial rows = g @ w1.T  (per-core rows) ----
    for _trep in range(tail_repeat):
      gm = spool.tile([128, CB, T, BL], F32, tag="gm", name="gm")
      nc.vector.tensor_scalar(
          out=gm[:], in0=stats[:], scalar1=-1.0 / N, scalar2=1.0,
          op0=OP.mult, op1=OP.add,
      )
      if tail_stage < 1:
          continue
      h1_ps = psum.tile([ROWS, CR], F32, tag="h1")
      for cb in range(CB):
          nc.tensor.matmul(
              h1_ps[:],
              gm[:, cb].rearrange("p t b -> p (t b)"),
              w1t_sb[:, cb, :],
              start=(cb == 0),
              stop=(cb == CB - 1),
          )
      h1_sb = tpool.tile([ROWS, CR], F32, tag="h1s")
      nc.vector.tensor_copy(h1_sb[:], h1_ps[:])

      # ---- AllGather local h1 rows -> all 64 batch rows on every core ----
      if tail_stage < 2:
          continue
      cc_in = dram.tile([ROWS, CR], F32)
      cc_out = dram.tile([TBALL, CR], F32)
      nc.sync.dma_start(cc_in[:], h1_sb[:])
      if single:
          for _slot in range(NCORES):
              nc.sync.dma_start(cc_out[ROWS * _slot:ROWS * (_slot + 1), :],
                                cc_in[:])
      else:
          nc.gpsimd.collective_compute(
              "AllGather", OP.bypass,
              replica_groups=[list(range(NCORES))],
              ins=[cc_in[:].opt()], outs=[cc_out[:].opt()],
          )
      h1_all = tpool.tile([TBALL, CR], F32, tag="h1a")
      nc.sync.dma_start(h1_all[:], cc_out[:])

      if tail_stage < 3:
          continue
      # ---- BN1 (stats over the 64 batch rows), in [j, tb] layout ----
      h1T_ps = psum.tile([CR, TBALL], F32, tag="tr", name="h1T_ps")
      nc.tensor.transpose(h1T_ps[:], h1_all[:], ident[:TBALL, :TBALL])
      h1T = tpool.tile([CR, TBALL], F32, tag="h1T", name="h1T")
      nc.vector.tensor_copy(h1T[:], h1T_ps[:])

      st6 = tpool.tile([CR, nc.vector.BN_STATS_DIM], F32, tag="st6", name="st6")
      nc.vector.bn_stats(st6[:], h1T[:])
      mv1 = tpool.tile([CR, nc.vector.BN_AGGR_DIM], F32, tag="mv1", name="mv1")
      nc.vector.bn_aggr(mv1[:], st6[:])
      std1 = tpool.tile([CR, 1], F32, tag="std1", name="std1")
      nc.scalar.activation(std1[:], mv1[:, 1:2], AF.Sqrt, bias=eps_sb[:CR])
      d1 = tpool.tile([CR, 1], F32, tag="d1", name="d1")
      nc.vector.reciprocal(d1[:], std1[:])
      nc.vector.tensor_mul(d1[:], d1[:], g1_sb[:])
      sh1 = tpool.tile([CR, 1], F32, tag="sh1", name="sh1")
      nc.vector.tensor_mul(sh1[:], mv1[:, 0:1], d1[:])
      nc.vector.tensor_sub(sh1[:], be1_sb[:], sh1[:])
      # write h1n columns permuted from gathered order (m, t, b) into
      # output order (t, m, b) so stage-B produces rows ready for one DMA
      h1nT = tpool.tile([CR, TBALL], F32, tag="h1nT", name="h1nT")
      h1nT_wr = h1nT[:].rearrange("j (t m b) -> j m t b", t=T, m=NCORES, b=BL)
      nc.vector.tensor_scalar(
          out=h1nT_wr, in0=h1T[:], scalar1=d1[:], scalar2=sh1[:],
          op0=OP.mult, op1=OP.add,
      )

      if tail_stage < 4:
          continue
      # ---- h2 = h1n @ w2.T  -> [64 rows, 512 ch] ----
      h2_ps = psum.tile([TBALL, C], F32, tag="h2", name="h2_ps")
      nc.tensor.matmul(h2_ps[:], h1nT[:], w2t_sb[:], start=True, stop=True)
      h2 = tpool.tile([TBALL, C], F32, tag="h2s", name="h2")
      nc.vector.tensor_copy(h2[:], h2_ps[:])
      # scale by 1/8 so sum(h2b^2) over the 64 rows is directly E[h2^2]
      h2b = tpool.tile([TBALL, C], BF16, tag="h2b", name="h2b")
      nc.scalar.activation(h2b[:], h2_ps[:], AF.Copy, scale=0.125)
      h2sq = tpool.tile([TBALL, C], BF16, tag="h2sq", name="h2sq")
      nc.vector.tensor_mul(h2sq[:], h2b[:], h2b[:])
      if tail_stage < 5:
          continue

      # ---- BN2: var over channels via one bf16 ones-matmul ----
      q2_ps = psum2.tile([1, C], F32, tag="q2", name="q2_ps")
      nc.tensor.matmul(q2_ps[:], ones_bf[:], h2sq[:], start=True, stop=True)
      var2 = tpool.tile([1, C], F32, tag="var2", name="var2")
      nc.vector.tensor_sub(var2[:], q2_ps[:], mu2sq[:])
      std2 = tpool.tile([1, C], F32, tag="std2", name="std2")
      nc.scalar.activation(std2[:], var2[:], AF.Sqrt, bias=eps_sb[:1])
      d2 = tpool.tile([1, C], F32, tag="d2", name="d2")
      nc.vector.reciprocal_approx_fast(d2[:], std2[:])
      nc.vector.tensor_mul(d2[:], d2[:], g2_sb[:])
      if tail_stage < 6:
          continue
      d2b = tpool.tile([TBALL, C], F32, tag="d2b", name="d2b")
      nc.gpsimd.partition_broadcast(d2b[:], d2[:])

      # out = (h2 - mu2) * d2 + beta2, rows already in output order
      o1 = tpool.tile([TBALL, C], F32, tag="o1", name="o1")
      nc.vector.tensor_sub(o1[:], h2[:], mu2b[:])
      o2 = tpool.tile([TBALL, C], F32, tag="o2", name="o2")
      nc.vector.tensor_mul(o2[:], o1[:], d2b[:])
      outf = tpool.tile([TBALL, C], F32, tag="outf", name="outf")
      nc.vector.tensor_add(outf[:], o2[:], be2b[:])
      if tail_stage < 7:
          continue

      nc.sync.dma_start(out[:].rearrange("t b c -> (t b) c"), outf[:])


_CACHE = {}


def _build(repeat=1, tail_repeat=1, single=False, tail_stage=99):
    key = ("nc", repeat, tail_repeat, single, tail_stage)
    if key in _CACHE:
        return _CACHE[key]
    from contextlib import ExitStack
    nc = bacc.Bacc("TRN2", target_bir_lowering=False, debug=False,
                   num_devices=1 if single else NCORES)
    with tile.TileContext(nc) as tc, ExitStack() as ctx:
        _emit(tc, ctx, repeat=repeat, tail_repeat=tail_repeat, single=single, tail_stage=tail_stage)
    nc.compile()
    _CACHE[key] = nc
    return nc


def make_in_maps(x, w1, gamma1, beta1, w2, gamma2, beta2):
    x = np.ascontiguousarray(np.asarray(x, dtype=np.float32))
    w1t = np.ascontiguousarray(np.asarray(w1, np.float32).T)
    w2t = np.ascontiguousarray(np.asarray(w2, np.float32).T)
    g1 = np.asarray(gamma1, np.float32).reshape(CR, 1)
    be1 = np.asarray(beta1, np.float32).reshape(CR, 1)
    g2 = np.asarray(gamma2, np.float32).reshape(1, C)
    be2 = np.asarray(beta2, np.float32).reshape(1, C)
    return [
        {
            "x": np.ascontiguousarray(x[:, BL * m:BL * (m + 1)]),
            "w1t": w1t, "w2t": w2t,
            "gamma1": g1, "beta1": be1,
            "gamma2": g2, "beta2": be2,
        }
        for m in range(NCORES)
    ]


def kernel(x, w1, b1, gamma1, beta1, w2, b2, gamma2, beta2):
    # b1/b2 cancel exactly inside the following batch-norms; unused.
    nc = _build()
    in_maps = make_in_maps(x, w1, gamma1, beta1, w2, gamma2, beta2)
    res = run_bass_kernel_spmd(nc, in_maps, core_ids=list(range(NCORES)))
    out = res.results[0]["out"]
    return np.asarray(out, np.float32).reshape(T, B, C, 1)



# revision 16
# speedup vs baseline: 59.7137x; 1.0144x over previous
"""Trainium2 Bass kernel for ChannelAttention-SNN (LIF -> GAP -> 1x1conv -> BN
-> 1x1conv -> BN).

Contract: kernel(**inputs) takes the FULL unsharded inputs (as produced by
setup_inputs) and returns the FULL output [T, B, C, 1] float32.

Strategy (hardcoded for T=4, B=16, C=512, N=1024, Cr=64, 8 cores):
  - Data-parallel over B: core m processes b in {2m, 2m+1}.
  - LIF scan is unrolled over T in "P-space": P_t = 2^t * v_pre_t, so
      P_t = P_{t-1} * m_{t-1} + 2^{t-1} * x_t,   spike_t <=> P_t >= 2^t,
    which folds the 1/tau decay into the (free) scale of the ScalarE cast
    fp32->bf16. Per timestep the VectorE does: mask = (P < theta) with a
    fused free-dim count (accum_out) that directly yields the GAP sums,
    a mask multiply, and an add.
  - The conv/BN tail runs in fp32 on the PE/DVE: per-core h1 partial rows
    [8, 64], one AllGather, then every core redundantly computes the
    batch-norm tail for all 64 rows and writes the full output (batch-stat
    all-reduce is subsumed by the gather; outputs are identical across
    cores).
"""

import numpy as np

import concourse.bacc as bacc
import concourse.bass as bass
import concourse.mybir as mybir
import concourse.tile as tile
from concourse.bass_utils import run_bass_kernel_spmd
from concourse.masks import make_identity

T, B, C, N, CR = 4, 16, 512, 1024, 64
NCORES = 8
BL = B // NCORES            # batch rows per core (2)
CB = C // 128               # 128-partition channel blocks (4)
ROWS = T * BL               # local (t, b) rows (8)
TBALL = T * B               # total batch rows for BN (64)
BN_EPS = 1e-5

F32 = mybir.dt.float32
BF16 = mybir.dt.bfloat16
OP = mybir.AluOpType
AF = mybir.ActivationFunctionType
AX = mybir.AxisListType


def _emit(tc, ctx, repeat=1, tail_repeat=1, single=False, tail_stage=99):
    nc = tc.nc
    x = nc.dram_tensor("x", [T, BL, C, N], F32, kind="ExternalInput").ap()
    w1t = nc.dram_tensor("w1t", [C, CR], F32, kind="ExternalInput").ap()
    w2t = nc.dram_tensor("w2t", [CR, C], F32, kind="ExternalInput").ap()
    g1 = nc.dram_tensor("gamma1", [CR, 1], F32, kind="ExternalInput").ap()
    be1 = nc.dram_tensor("beta1", [CR, 1], F32, kind="ExternalInput").ap()
    g2 = nc.dram_tensor("gamma2", [1, C], F32, kind="ExternalInput").ap()
    be2 = nc.dram_tensor("beta2", [1, C], F32, kind="ExternalInput").ap()
    out = nc.dram_tensor("out", [T, B, C], F32, kind="ExternalOutput").ap()

    consts = ctx.enter_context(tc.tile_pool(name="consts", bufs=1))
    xpool = ctx.enter_context(tc.tile_pool(name="xp", bufs=5))
    ypool = ctx.enter_context(tc.tile_pool(name="yp", bufs=3))
    qpool = ctx.enter_context(tc.tile_pool(name="qp", bufs=3))
    mpool = ctx.enter_context(tc.tile_pool(name="mp", bufs=8))
    spool = ctx.enter_context(tc.tile_pool(name="sp", bufs=1))
    tpool = ctx.enter_context(tc.tile_pool(name="tp", bufs=2))
    psum = ctx.enter_context(tc.tile_pool(name="ps", bufs=1, space="PSUM"))
    psum2 = ctx.enter_context(tc.tile_pool(name="ps2", bufs=1, space="PSUM"))
    dram = ctx.enter_context(tc.tile_pool(name="dr", bufs=1, space="DRAM"))

    # ---- constants / weights (overlaps with the streaming phase) ----
    ident = consts.tile([128, 128], F32)
    make_identity(nc, ident)
    w1t_sb = consts.tile([128, CB, CR], F32)
    for cb in range(CB):
        nc.sync.dma_start(w1t_sb[:, cb, :], w1t[cb * 128:(cb + 1) * 128, :])
    w2t_sb = consts.tile([CR, C], F32)
    nc.sync.dma_start(w2t_sb[:], w2t[:])
    g1_sb = consts.tile([CR, 1], F32)
    nc.sync.dma_start(g1_sb[:], g1[:])
    be1_sb = consts.tile([CR, 1], F32)
    nc.sync.dma_start(be1_sb[:], be1[:])
    g2_sb = consts.tile([1, C], F32)
    nc.sync.dma_start(g2_sb[:], g2[:])
    be2_sb = consts.tile([1, C], F32)
    nc.sync.dma_start(be2_sb[:], be2[:])
    # BN1 guarantees mean(h1n) == beta1, so BN2's channel mean is known
    # ahead of time: mu2 = beta1 @ w2.T (+b2, which cancels).
    mu2_ps = psum2.tile([1, C], F32, tag="mu2p", name="mu2_ps")
    nc.tensor.matmul(mu2_ps[:], be1_sb[:], w2t_sb[:], start=True, stop=True)
    mu2row = consts.tile([1, C], F32)
    nc.vector.tensor_scalar_mul(mu2row[:], mu2_ps[:], 1.0 / 1.0)
    mu2sq = consts.tile([1, C], F32)
    nc.vector.tensor_mul(mu2sq[:], mu2row[:], mu2row[:])
    mu2b = consts.tile([TBALL, C], F32)
    nc.gpsimd.partition_broadcast(mu2b[:], mu2row[:])
    be2b = consts.tile([TBALL, C], F32)
    nc.gpsimd.partition_broadcast(be2b[:], be2_sb[:])
    ones_sb = consts.tile([TBALL, 1], F32)
    nc.vector.memset(ones_sb[:], 1.0)
    eps_sb = consts.tile([128, 1], F32)
    nc.vector.memset(eps_sb[:], BN_EPS)
    warm_sb = consts.tile([128, 1], F32)
    # warm the Sqrt activation table during the streaming phase
    nc.scalar.activation(warm_sb[:], eps_sb[:], AF.Sqrt, bias=eps_sb[:])
    ones_bf = consts.tile([TBALL, 1], BF16)
    nc.vector.memset(ones_bf[:], 1.0)

    # ---- streaming LIF + GAP ----
    # stats[:, cb, t, b] = sum_n (P_t < theta_t)  (count of NON-spikes)
    stats = spool.tile([128, CB, T, BL], F32)
    pstate = [spool.tile([128, BL, N], BF16, tag=f"P{cb}", name=f"P{cb}")
              for cb in range(CB)]
    masks = [None] * CB

    for _rep in range(repeat):
      for t in range(T):
          for cb in range(CB):
              P = pstate[cb]
              xt = xpool.tile([128, BL, N], F32)
              src = x[t, :, cb * 128:(cb + 1) * 128, :].rearrange("b c n -> c b n")
              nc.sync.dma_start(xt[:], src)
              if t == 0:
                  # P_1 = x_1 (cast to bf16)
                  nc.scalar.activation(P[:], xt[:], AF.Copy, scale=1.0)
              else:
                  y = ypool.tile([128, BL, N], BF16)
                  nc.scalar.activation(y[:], xt[:], AF.Copy, scale=float(2 ** t))
                  q = qpool.tile([128, BL, N], BF16)
                  nc.vector.tensor_mul(q[:], P[:], masks[cb][:])
                  nc.vector.tensor_add(P[:], q[:], y[:])
              m = mpool.tile([128, BL, N], BF16)
              theta = float(2 ** (t + 1))
              for b in range(BL):
                  nc.vector.tensor_scalar(
                      out=m[:, b, :],
                      in0=P[:, b, :],
                      scalar1=theta,
                      scalar2=None,
                      op0=OP.is_lt,
                      op1=OP.add,
                      accum_out=stats[:, cb, t, b:b + 1],
                  )
              masks[cb] = m

    # ---- g = 1 - stats/N ; h1 partial rows = g @ w1.T  (per-core rows) ----
    for _trep in range(tail_repeat):
      gm = spool.tile([128, CB, T, BL], F32, tag="gm", name="gm")
      nc.vector.tensor_scalar(
          out=gm[:], in0=stats[:], scalar1=-1.0 / N, scalar2=1.0,
          op0=OP.mult, op1=OP.add,
      )
      if tail_stage < 1:
          continue
      h1_ps = psum.tile([ROWS, CR], F32, tag="h1")
      for cb in range(CB):
          nc.tensor.matmul(
              h1_ps[:],
              gm[:, cb].rearrange("p t b -> p (t b)"),
              w1t_sb[:, cb, :],
              start=(cb == 0),
              stop=(cb == CB - 1),
          )
      h1_sb = tpool.tile([ROWS, CR], F32, tag="h1s")
      nc.vector.tensor_copy(h1_sb[:], h1_ps[:])

      # ---- AllGather local h1 rows -> all 64 batch rows on every core ----
      if tail_stage < 2:
          continue
      cc_in = dram.tile([ROWS, CR], F32)
      cc_out = dram.tile([TBALL, CR], F32)
      nc.sync.dma_start(cc_in[:], h1_sb[:])
      if single:
          for _slot in range(NCORES):
              nc.sync.dma_start(cc_out[ROWS * _slot:ROWS * (_slot + 1), :],
                                cc_in[:])
      else:
          nc.gpsimd.collective_compute(
              "AllGather", OP.bypass,
              replica_groups=[list(range(NCORES))],
              ins=[cc_in[:].opt()], outs=[cc_out[:].opt()],
          )
      h1_all = tpool.tile([TBALL, CR], F32, tag="h1a")
      nc.sync.dma_start(h1_all[:], cc_out[:])

      if tail_stage < 3:
          continue
      # ---- BN1 (stats over the 64 batch rows), in [j, tb] layout ----
      h1T_ps = psum.tile([CR, TBALL], F32, tag="tr", name="h1T_ps")
      nc.tensor.transpose(h1T_ps[:], h1_all[:], ident[:TBALL, :TBALL])
      h1T = tpool.tile([CR, TBALL], F32, tag="h1T", name="h1T")
      nc.vector.tensor_copy(h1T[:], h1T_ps[:])

      st6 = tpool.tile([CR, nc.vector.BN_STATS_DIM], F32, tag="st6", name="st6")
      nc.vector.bn_stats(st6[:], h1T[:])
      mv1 = tpool.tile([CR, nc.vector.BN_AGGR_DIM], F32, tag="mv1", name="mv1")
      nc.vector.bn_aggr(mv1[:], st6[:])
      std1 = tpool.tile([CR, 1], F32, tag="std1", name="std1")
      nc.scalar.activation(std1[:], mv1[:, 1:2], AF.Sqrt, bias=eps_sb[:CR])
      d1 = tpool.tile([CR, 1], F32, tag="d1", name="d1")
      nc.vector.reciprocal(d1[:], std1[:])
      nc.vector.tensor_mul(d1[:], d1[:], g1_sb[:])
      sh1 = tpool.tile([CR, 1], F32, tag="sh1", name="sh1")
      nc.vector.tensor_mul(sh1[:], mv1[:, 0:1], d1[:])
      nc.vector.tensor_sub(sh1[:], be1_sb[:], sh1[:])
      # write h1n columns permuted from gathered order (m, t, b) into
      # output order (t, m, b) so stage-B produces rows ready for one DMA
      h1nT = tpool.tile([CR, TBALL], F32, tag="h1nT", name="h1nT")
      h1nT_wr = h1nT[:].rearrange("j (t m b) -> j m t b", t=T, m=NCORES, b=BL)
      nc.vector.tensor_scalar(
          out=h1nT_wr, in0=h1T[:], scalar1=d1[:], scalar2=sh1[:],
          op0=OP.mult, op1=OP.add,
      )

      if tail_stage < 4:
          continue
      # ---- h2 = h1n @ w2.T  -> [64 rows, 512 ch] ----
      h2_ps = psum.tile([TBALL, C], F32, tag="h2", name="h2_ps")
      nc.tensor.matmul(h2_ps[:], h1nT[:], w2t_sb[:], start=True, stop=True)
      h2 = tpool.tile([TBALL, C], F32, tag="h2s", name="h2")
      nc.vector.tensor_copy(h2[:], h2_ps[:])
      # scale by 1/8 so sum(h2b^2) over the 64 rows is directly E[h2^2]
      h2b = tpool.tile([TBALL, C], BF16, tag="h2b", name="h2b")
      nc.scalar.activation(h2b[:], h2_ps[:], AF.Copy, scale=0.125)
      h2sq = tpool.tile([TBALL, C], BF16, tag="h2sq", name="h2sq")
      nc.vector.tensor_mul(h2sq[:], h2b[:], h2b[:])
      if tail_stage < 5:
          continue

      # ---- BN2: var over channels via one bf16 ones-matmul ----
      q2_ps = psum2.tile([1, C], F32, tag="q2", name="q2_ps")
      nc.tensor.matmul(q2_ps[:], ones_bf[:], h2sq[:], start=True, stop=True)
      var2 = tpool.tile([1, C], F32, tag="var2", name="var2")
      nc.vector.tensor_sub(var2[:], q2_ps[:], mu2sq[:])
      std2 = tpool.tile([1, C], F32, tag="std2", name="std2")
      nc.scalar.activation(std2[:], var2[:], AF.Sqrt, bias=eps_sb[:1])
      d2 = tpool.tile([1, C], F32, tag="d2", name="d2")
      nc.vector.reciprocal_approx_fast(d2[:], std2[:])
      nc.vector.tensor_mul(d2[:], d2[:], g2_sb[:])
      if tail_stage < 6:
          continue
      d2b = tpool.tile([TBALL, C], F32, tag="d2b", name="d2b")
      nc.gpsimd.partition_broadcast(d2b[:], d2[:])

      # out = (h2 - mu2) * d2 + beta2, rows already in output order
      o1 = tpool.tile([TBALL, C], F32, tag="o1", name="o1")
      nc.vector.tensor_sub(o1[:], h2[:], mu2b[:])
      o2 = tpool.tile([TBALL, C], F32, tag="o2", name="o2")
      nc.vector.tensor_mul(o2[:], o1[:], d2b[:])
      outf = tpool.tile([TBALL, C], F32, tag="outf", name="outf")
      nc.vector.tensor_add(outf[:], o2[:], be2b[:])
      if tail_stage < 7:
          continue

      nc.sync.dma_start(out[:].rearrange("t b c -> (t b) c"), outf[:])


_CACHE = {}


def _build(repeat=1, tail_repeat=1, single=False, tail_stage=99):
    key = ("nc", repeat, tail_repeat, single, tail_stage)
    if key in _CACHE:
        return _CACHE[key]
    from contextlib import ExitStack
    nc = bacc.Bacc("TRN2", target_bir_lowering=False, debug=False,
                   num_devices=1 if single else NCORES)
    with tile.TileContext(nc) as tc, ExitStack() as ctx:
        _emit(tc, ctx, repeat=repeat, tail_repeat=tail_repeat, single=single, tail_stage=tail_stage)
    nc.compile()
    _CACHE[key] = nc
    return nc


def make_in_maps(x, w1, gamma1, beta1, w2, gamma2, beta2):
    x = np.ascontiguousarray(np.asarray(x, dtype=np.float32))
    w1t = np.ascontiguousarray(np.asarray(w1, np.float32).T)
    w2t = np.ascontiguousarray(np.asarray(w2, np.float32).T)
    g1 = np.asarray(gamma1, np.float32).reshape(CR, 1)
    be1 = np.asarray(beta1, np.float32).reshape(CR, 1)
    g2 = np.asarray(gamma2, np.float32).reshape(1, C)
    be2 = np.asarray(beta2, np.float32).reshape(1, C)
    return [
        {
            "x": np.ascontiguousarray(x[:, BL * m:BL * (m + 1)]),
            "w1t": w1t, "w2t": w2t,
            "gamma1": g1, "beta1": be1,
            "gamma2": g2, "beta2": be2,
        }
        for m in range(NCORES)
    ]


def kernel(x, w1, b1, gamma1, beta1, w2, b2, gamma2, beta2):
    # b1/b2 cancel exactly inside the following batch-norms; unused.
    nc = _build()
    in_maps = make_in_maps(x, w1, gamma1, beta1, w2, gamma2, beta2)
    res = run_bass_kernel_spmd(nc, in_maps, core_ids=list(range(NCORES)))
    out = res.results[0]["out"]
    return np.asarray(out, np.float32).reshape(T, B, C, 1)

